# revision 35
# baseline (speedup 1.0000x reference)
"""Gated multi-head self-attention on 8 Trainium2 NeuronCores.

Sharding: 16 heads / 8 cores = 2 heads per core. Each core computes its two
heads end-to-end (QKV projection, attention, per-head norm, output
projection) and writes a partial [B*T, D] output in bf16; the host sums the
8 partials and adds the head-summed output bias.

Per-core algorithm (heads h0, h1), bf16 matmul datapath, f32 PSUM:
  QT/KT[128, 4096]   = W_{q,k}.T @ x.T + b  (heads stacked on partitions;
                                             softmax scale folded into W_q)
  V'[128s, j, h, 65] = x @ W_v | 1          (direct projection per 128-token
                                             tile; col 64 = ones for denom)
  S^T[s, q]          = KT.T @ QT            (two PE quadrant matmuls per j)
  P                  = exp(S^T)             (Act engine, PSUM -> bf16 SBUF)
  po[65, q]          = V'.T @ P             (row 64 = softmax denominators)
  osm                = (po * 1/den_row) + b_v
  denom_h            = max(mean_t ||osm[:, t]||, 1e-5)   via exp(0.5 ln x)
  out                = osm.T @ (W_o * g/16 * 1/denom_h)  (contract-128 packed)

Scheduling: the Act engine (exp over the full [T,T] scores) and PE are the
two near-saturated engines; V' tiles and K/Q projections of the second
batch's slabs are emitted inside the attention j-loops as PE filler, and
the per-chunk norm reductions ride along two chunks behind.
"""

import sys

sys.path.insert(0, "/opt/trn_rl_repo")

import contextlib

import numpy as np

import concourse.bacc as bacc
import concourse.mybir as mybir
import concourse.tile as tile
from concourse.bass_utils import run_bass_kernel_spmd

f32 = mybir.dt.float32
bf16 = mybir.dt.bfloat16
AF = mybir.ActivationFunctionType
ALU = mybir.AluOpType

B, T, D, H, HD = 2, 2048, 1024, 16, 64
NCORES = 8
HPC = H // NCORES  # heads per core = 2
NT = B * T         # 4096 tokens
SCALE = 1.0 / np.sqrt(HD)  # 0.125 (exact in bf16)

_BUILD_CACHE = {}


def _steered_act_tables(arch):
    """Same list/order as act_info.json, but Exp/Ln only appear in the one
    set containing both, so the table-load pass hoists a single load."""
    from concourse.hw_specs import get_activation_tables
    tabs = get_activation_tables(arch)
    both = [n for n, fns in tabs.items()
            if AF.Exp in fns and AF.Ln in fns]
    if not both:
        return tabs
    keep = both[0]
    out = {}
    for n, fns in tabs.items():
        if n == keep:
            out[n] = fns
        else:
            out[n] = fns - {AF.Exp, AF.Ln}
    return out


def _build(with_mask: bool, repeat: int = 1, unroll: int = 1):
    import unittest.mock as _mock
    with _mock.patch.object(bacc, "get_activation_tables",
                            _steered_act_tables):
        return _build_inner(with_mask, repeat, unroll)


def _build_inner(with_mask: bool, repeat: int = 1, unroll: int = 1):
    nc = bacc.Bacc(None, target_bir_lowering=False)

    xb = nc.declare_dram_parameter("xb", [8, 128, 8, 512], bf16, isOutput=False)
    wqkv = nc.declare_dram_parameter("wqkv", [128, 3, 8, 128], bf16, isOutput=False)
    bqk = nc.declare_dram_parameter("bqk", [128, 2], f32, isOutput=False)
    bv128 = nc.declare_dram_parameter("bv128", [128, 1], f32, isOutput=False)
    wo = nc.declare_dram_parameter("wo", [128, D], bf16, isOutput=False)
    outp = nc.declare_dram_parameter("outp", [32, 128, 2, 512], bf16, isOutput=True)
    if with_mask:
        maskT = nc.declare_dram_parameter("maskT", [T, T], f32, isOutput=False)

    NJ = T // 128  # 16 key tiles per batch

    with tile.TileContext(nc) as tc, contextlib.ExitStack() as ctx:
        wp = ctx.enter_context(tc.tile_pool(name="wp", bufs=1))
        xsp = ctx.enter_context(tc.tile_pool(name="xsp", bufs=8))
        pp = ctx.enter_context(tc.tile_pool(name="pp", bufs=5))
        o65p = ctx.enter_context(tc.tile_pool(name="o65p", bufs=3))
        rowp = ctx.enter_context(tc.tile_pool(name="rowp", bufs=3))
        bcp = ctx.enter_context(tc.tile_pool(name="bcp", bufs=3))
        stp = ctx.enter_context(tc.tile_pool(name="stp", bufs=3))
        sqp = ctx.enter_context(tc.tile_pool(name="sqp", bufs=3))
        lnp = ctx.enter_context(tc.tile_pool(name="lnp", bufs=2))
        scp = ctx.enter_context(tc.tile_pool(name="scp", bufs=1))
        obp = ctx.enter_context(tc.tile_pool(name="obp", bufs=6))
        if with_mask:
            mp = ctx.enter_context(tc.tile_pool(name="mp", bufs=2))
            tmpp = ctx.enter_context(tc.tile_pool(name="tmpp", bufs=2))
        psA = ctx.enter_context(tc.tile_pool(name="psA", bufs=2, space="PSUM"))
        psB = ctx.enter_context(tc.tile_pool(name="psB", bufs=1, space="PSUM"))
        psC = ctx.enter_context(tc.tile_pool(name="psC", bufs=2, space="PSUM"))

        # ---- persistent weights / constants (outside repeat loop) ----
        wqkv_sb = wp.tile([128, 3, 8, 128], bf16)
        nc.sync.dma_start(out=wqkv_sb[:], in_=wqkv.ap())
        bqk_sb = wp.tile([128, 2], f32)
        nc.sync.dma_start(out=bqk_sb[:], in_=bqk.ap())
        bv_sb = wp.tile([128, 1], f32)
        nc.sync.dma_start(out=bv_sb[:], in_=bv128.ap())
        wo_sb = wp.tile([128, D], bf16)
        nc.sync.dma_start(out=wo_sb[:], in_=wo.ap())

        ones128 = wp.tile([128, 1], bf16)
        nc.vector.memset(ones128[:], 1.0)
        ind33 = wp.tile([33, 2, 128], f32)
        nc.vector.memset(ind33[:], 0.0)
        nc.vector.memset(ind33[0:1, 0, 0:64], 1.0)
        nc.vector.memset(ind33[32:33, 1, 64:128], 1.0)

        # V' [s-part, j-tile, head, 66]: cols 0:64 = V, col 64 = ones
        Vp = wp.tile([128, NT // 128, HPC, 66], bf16)
        nc.vector.memset(Vp[:, :, :, 64:65], 1.0)

        QT = wp.tile([128, NT], bf16)
        KT = wp.tile([128, NT], bf16)
        osm = wp.tile([128, NT], bf16)
        wo_s = wp.tile([128, D], bf16)
        tparts = wp.tile([33, 8], f32)

        xs_tiles = {}

        def dma_xs(c8):
            xs = xsp.tile([128, 8, 512], bf16, tag="xs", name=f"xs{c8}")
            nc.sync.dma_start(out=xs[:], in_=xb.ap()[c8])
            xs_tiles[c8] = xs

        def kq_proj(c8, p, tag="c"):
            """Full 8-matmul projection of slab c8 into QT (p=0) / KT (p=1)."""
            xs = xs_tiles[c8]
            cols = slice(c8 * 512, (c8 + 1) * 512)
            dst = QT if p == 0 else KT
            ps = psC.tile([128, 512], f32, tag=tag, name="ps_kq")
            for dc in range(8):
                nc.tensor.matmul(ps[:], wqkv_sb[:, p, dc, :], xs[:, dc, :],
                                 start=(dc == 0), stop=(dc == 7))
            nc.vector.tensor_scalar_add(dst[:, cols], ps[:], bqk_sb[:, p:p + 1])

        def vp_tile(b, j):
            """V' projection for key tile j of batch b (one 128-token tile)."""
            c8 = b * 4 + j // 4
            s4 = j % 4
            xs = xs_tiles[c8]
            psv = psC.tile([128, 128], f32, tag="c", name="ps_v")
            for dc in range(8):
                nc.tensor.matmul(psv[:],
                                 xs[:, dc, s4 * 128:(s4 + 1) * 128],
                                 wqkv_sb[:, 2, dc, :],
                                 start=(dc == 0), stop=(dc == 7))
            nc.vector.tensor_copy(Vp[:, b * NJ + j, :, 0:64], psv[:])

        # staged K/Q filler: one matmul per call, drain on the 9th call
        class KQSpread:
            def __init__(self, c8, p):
                self.c8, self.p = c8, p
                self.ps = None
                self.dc = 0

            def step(self):
                if self.dc >= 8:
                    return
                if self.ps is None:
                    self.ps = psC.tile([128, 512], f32, tag="c", name="ps_sp")
                xs = xs_tiles[self.c8]
                nc.tensor.matmul(self.ps[:], wqkv_sb[:, self.p, self.dc, :],
                                 xs[:, self.dc, :],
                                 start=(self.dc == 0), stop=(self.dc == 7))
                self.dc += 1
                if self.dc == 8:
                    cols = slice(self.c8 * 512, (self.c8 + 1) * 512)
                    dst = QT if self.p == 0 else KT
                    nc.vector.tensor_scalar_add(dst[:, cols], self.ps[:],
                                                bqk_sb[:, self.p:self.p + 1])

        def pn_norm(b, qc):
            """Both heads' sum-of-squares column sums + sqrt-sum via ln/exp."""
            sq0, sq1 = sq_tiles[(b, qc)]
            pn = psC.tile([33, 512], f32, tag="c", name="pn")
            nc.tensor.matmul(pn[0:1, :], ones128[0:64, :], sq0[:],
                             start=True, stop=True, tile_position=(0, 0))
            nc.tensor.matmul(pn[32:33, :], ones128[0:64, :], sq1[:],
                             start=True, stop=True, tile_position=(0, 32))
            lnt = lnp.tile([33, 512], f32, tag="ln")
            nc.scalar.activation(lnt[:], pn[:], AF.Ln)
            sc = lnp.tile([33, 512], f32, tag="ln")
            nc.scalar.activation(sc[:], lnt[:], AF.Exp, scale=0.5,
                                 accum_out=tparts[:, b * 4 + qc:b * 4 + qc + 1])

        sq_tiles = {}

        def attn_qc(b, qc, fill=None):
            """One 512-query chunk of attention for batch b, both heads.

            fill: optional list of 16 callables; fill[j]() is emitted after
            iteration j's attention matmuls as PE filler work.
            """
            qcols = slice(b * T + qc * 512, b * T + (qc + 1) * 512)
            po = psB.tile([65, 1024], f32, tag="po", name="po")
            prev_pe = None

            def av(j, pe):
                for h in range(HPC):
                    nc.tensor.matmul(po[:, h * 512:(h + 1) * 512],
                                     Vp[:, b * NJ + j, h, 0:65],
                                     pe[:, h * 512:(h + 1) * 512],
                                     start=(j == 0), stop=(j == NJ - 1))

            for j in range(NJ):
                scols = slice(b * T + j * 128, b * T + (j + 1) * 128)
                s2 = psA.tile([128, 1024], f32, tag="s2", name="s2")
                for h in range(HPC):
                    nc.tensor.matmul(s2[:, h * 512:(h + 1) * 512],
                                     KT[h * 64:(h + 1) * 64, scols],
                                     QT[h * 64:(h + 1) * 64, qcols],
                                     start=True, stop=True,
                                     tile_position=(h * 64, 0))
                pe = pp.tile([128, 1024], bf16, tag="p")
                if with_mask:
                    mt = mp.tile([128, 512], f32, tag="m")
                    nc.sync.dma_start(
                        out=mt[:],
                        in_=maskT.ap()[j * 128:(j + 1) * 128,
                                       qc * 512:(qc + 1) * 512])
                    tmp = tmpp.tile([128, 1024], f32, tag="tmp")
                    for h in range(HPC):
                        nc.vector.tensor_tensor(
                            tmp[:, h * 512:(h + 1) * 512],
                            s2[:, h * 512:(h + 1) * 512], mt[:], op=ALU.add)
                    nc.scalar.activation(pe[:], tmp[:], AF.Exp)
                else:
                    nc.scalar.activation(pe[:], s2[:], AF.Exp)
                if prev_pe is not None:
                    av(j - 1, prev_pe)
                prev_pe = pe
                if fill is not None:
                    fill[j]()
            av(NJ - 1, prev_pe)

            # epilogue: normalize by softmax denominators, add b_v
            o65 = o65p.tile([65, 1024], f32, tag="o65")
            nc.vector.tensor_copy(o65[:], po[:])
            drow = rowp.tile([1, 1024], f32, tag="row")
            nc.sync.dma_start(out=drow[:], in_=o65[64:65, :])
            rrow = rowp.tile([1, 1024], f32, tag="row")
            nc.vector.reciprocal(rrow[:], drow[:])
            bc = bcp.tile([64, 1024], f32, tag="bc")
            nc.gpsimd.partition_broadcast(bc[:], rrow[:])
            t0 = stp.tile([64, 512], f32, tag="t0")
            nc.vector.tensor_tensor(t0[:], o65[0:64, 0:512],
                                    bc[:, 0:512], op=ALU.mult)
            nc.vector.tensor_scalar_add(osm[0:64, qcols], t0[:],
                                        bv_sb[0:64, 0:1])
            t1 = stp.tile([64, 512], f32, tag="t0")
            nc.vector.tensor_tensor(t1[:], o65[0:64, 512:1024],
                                    bc[:, 512:1024], op=ALU.mult)
            st = stp.tile([64, 512], bf16, tag="st")
            nc.vector.tensor_scalar_add(st[:], t1[:], bv_sb[64:128, 0:1])
            nc.gpsimd.dma_start(out=osm[64:128, qcols], in_=st[:])
            # squared-osm tiles per head for the (deferred) norm reduction;
            # sq1 comes from the staging tile so the norm path does not wait
            # on the h1 restack DMA
            sq0 = sqp.tile([64, 512], bf16, tag="sq0", name=f"sqa{b}{qc}")
            nc.vector.tensor_tensor(sq0[:], osm[0:64, qcols],
                                    osm[0:64, qcols], op=ALU.mult)
            sq1 = sqp.tile([64, 512], bf16, tag="sq1", name=f"sqb{b}{qc}")
            nc.vector.tensor_tensor(sq1[:], st[:], st[:], op=ALU.mult)
            sq_tiles[(b, qc)] = (sq0, sq1)

        def finalize():
            # per-head total norms -> denominators -> fold into W_o
            tot33 = scp.tile([33, 2], f32, tag="t0", name="tot33")
            dm0 = lnp.tile([1, 8], f32, tag="ln", name="dm0")
            nc.vector.tensor_scalar(dm0[:], tparts[0:1, :], 1.0, 0.0,
                                    op0=ALU.mult, op1=ALU.add,
                                    accum_out=tot33[0:1, 0:1])
            dm1 = lnp.tile([33, 8], f32, tag="ln", name="dm1")
            nc.vector.tensor_scalar(dm1[32:33, :], tparts[32:33, :], 1.0,
                                    0.0, op0=ALU.mult, op1=ALU.add,
                                    accum_out=tot33[32:33, 1:2])
            den33 = scp.tile([33, 2], f32, tag="t1", name="den33")
            nc.vector.tensor_scalar(den33[:], tot33[:], 1.0 / NT, 1e-5,
                                    op0=ALU.mult, op1=ALU.max)
            rcp33 = scp.tile([33, 2], f32, tag="t2", name="rcp33")
            nc.vector.reciprocal(rcp33[:], den33[:])
            psI = psC.tile([128, 1], f32, tag="c", name="psI")
            nc.tensor.matmul(psI[:], ind33[0:1, 0, :], rcp33[0:1, 0:1],
                             start=True, stop=False, tile_position=(0, 0))
            nc.tensor.matmul(psI[:], ind33[32:33, 1, :], rcp33[32:33, 1:2],
                             start=False, stop=True, tile_position=(32, 0))
            nc.vector.tensor_scalar(wo_s[:], wo_sb[:], psI[:, 0:1], None,
                                    op0=ALU.mult)
            # output projection: contract 128 (both heads) per token tile;
            # [128, 1024] psum tiles rotate through psA/psA/psB (s2/po slots
            # are free by now) for a 3-deep pipeline
            for t2 in range(NT // 256):
                osb = obp.tile([128, 2, 2, 512], bf16, tag="ob")
                for ti in range(2):
                    t = t2 * 2 + ti
                    trows = slice(t * 128, (t + 1) * 128)
                    pool, ptag = ((psA, "s2"), (psA, "s2"),
                                  (psB, "po"))[t % 3]
                    ppj = pool.tile([128, 1024], f32, tag=ptag, name="ps_p")
                    for dchunk in range(2):
                        dcols = slice(dchunk * 512, (dchunk + 1) * 512)
                        nc.tensor.matmul(ppj[:, dchunk * 512:
                                             (dchunk + 1) * 512],
                                         osm[:, trows], wo_s[:, dcols],
                                         start=True, stop=True)
                    if ti % 2 == 0:
                        nc.vector.tensor_copy(osb[:, ti, :, :], ppj[:])
                    else:
                        nc.scalar.copy(osb[:, ti, :, :], ppj[:])
                nc.sync.dma_start(
                    out=outp.ap()[t2 * 2:t2 * 2 + 2].rearrange(
                        "t p d c -> p t d c"),
                    in_=osb[:])

        def _emit_all():
            sq_tiles.clear()
            xs_tiles.clear()
            for c8 in (0, 1, 2, 3, 7, 4, 5, 6):
                dma_xs(c8)
            # head: K of b0 slabs + Q for chunks (0,0)/(0,1), plus the two
            # projections that have no spread slot later (Q5, K7)
            for c8 in range(4):
                kq_proj(c8, 1)
            kq_proj(0, 0)
            kq_proj(1, 0)
            kq_proj(5, 0)
            kq_proj(7, 1)
            nop = lambda: None

            def spread_fill(a, b_, norms):
                """fill list: a.step at j<8, b_.step at j>=8, norms bundled."""
                fill = []
                for j in range(NJ):
                    base = a if j < 8 else b_
                    nrm = dict(norms).get(j)
                    if nrm is not None:
                        bb, qq = nrm
                        fill.append(lambda s=base, b2=bb, q2=qq:
                                    (s.step() if s else None,
                                     pn_norm(b2, q2)))
                    else:
                        fill.append(base.step if base else nop)
                return fill

            def vp_chain_fill(b, chain, steps_per_j):
                """V' tiles each j plus steps from a sequence of KQSpreads."""
                def mk(j):
                    def f():
                        vp_tile(b, j)
                        idx = (j * steps_per_j) // 8
                        for _ in range(steps_per_j):
                            if idx < len(chain):
                                chain[idx].step()
                    return f
                return [mk(j) for j in range(NJ)]

            # b0 chunks: V' of b0 in qc0; Q2-4 / K4-6 spread over qc1-3
            attn_qc(0, 0, vp_chain_fill(0, [], 0))
            attn_qc(0, 1, spread_fill(KQSpread(2, 0), KQSpread(4, 1), {}))
            attn_qc(0, 2, spread_fill(KQSpread(3, 0), KQSpread(5, 1),
                                      {10: (0, 0)}))
            attn_qc(0, 3, spread_fill(KQSpread(4, 0), KQSpread(6, 1),
                                      {10: (0, 1)}))
            # b1 chunks: V' of b1 in qc0; Q6/Q7 + remaining norms ride along
            attn_qc(1, 0, vp_chain_fill(1, [], 0))
            attn_qc(1, 1, spread_fill(KQSpread(6, 0), None,
                                      {9: (0, 2), 12: (0, 3)}))
            attn_qc(1, 2, spread_fill(KQSpread(7, 0), None,
                                      {9: (1, 0), 12: (1, 1)}))
            attn_qc(1, 3, spread_fill(None, None, {12: (1, 2)}))
            pn_norm(1, 3)
            finalize()

        if repeat > 1:
            # unroll inside the hardware loop: divides the per-iteration
            # all-engine barrier cost and lets body k+1's input DMAs overlap
            # body k's projection/output tail
            with tc.For_i(0, repeat // unroll, 1):
                for _ in range(unroll):
                    _emit_all()
            for _ in range(repeat % unroll):
                _emit_all()
        else:
            _emit_all()

    nc.compile()
    return nc


def _get_nc(with_mask: bool):
    key = with_mask
    if key not in _BUILD_CACHE:
        _BUILD_CACHE[key] = _build(with_mask)
    return _BUILD_CACHE[key]


def build_in_maps(hidden_states, attn_mask, W_q, b_q, W_k, b_k, W_v, b_v,
                  W_o, b_o, gate, with_mask):
    import ml_dtypes
    bfl = ml_dtypes.bfloat16

    x = np.asarray(hidden_states, np.float32).reshape(NT, D)
    # [c8, p, dc, tt]: xb[c8][p, dc, tt] = x[c8*512+tt, dc*128+p]
    xb_all = np.ascontiguousarray(
        x.T.reshape(8, 128, 8, 512).transpose(2, 1, 0, 3)).astype(bfl)
    g = np.clip(np.asarray(gate, np.float32), 0.0, 1.0)

    in_maps = []
    for c in range(NCORES):
        h0, h1 = c * HPC, c * HPC + 1
        wq = np.concatenate([W_q[h0], W_q[h1]], axis=1) * SCALE  # [D, 128]
        wk = np.concatenate([W_k[h0], W_k[h1]], axis=1)
        wv = np.concatenate([W_v[h0], W_v[h1]], axis=1)
        # [p, 3, dc, m]
        wqkv_c = np.ascontiguousarray(
            np.stack([w.reshape(8, 128, 128).transpose(1, 0, 2)
                      for w in (wq, wk, wv)], axis=1)).astype(bfl)
        bqk_c = np.ascontiguousarray(np.stack(
            [np.concatenate([b_q[h0], b_q[h1]]) * SCALE,
             np.concatenate([b_k[h0], b_k[h1]])], axis=1)).astype(np.float32)
        bv_c = np.ascontiguousarray(
            np.concatenate([b_v[h0], b_v[h1]])[:, None]).astype(np.float32)
        wo_c = np.ascontiguousarray(np.concatenate(
            [W_o[h0] * (g[h0] / H), W_o[h1] * (g[h1] / H)],
            axis=0)).astype(bfl)
        m = dict(xb=xb_all, wqkv=wqkv_c, bqk=bqk_c, bv128=bv_c, wo=wo_c)
        if with_mask:
            m["maskT"] = np.ascontiguousarray(
                np.asarray(attn_mask, np.float32).T)
        in_maps.append(m)
    return in_maps


def kernel(hidden_states, attn_mask, W_q, b_q, W_k, b_k, W_v, b_v, W_o, b_o,
           gate):
    attn_mask = np.asarray(attn_mask, dtype=np.float32)
    W_q, b_q = np.asarray(W_q, np.float32), np.asarray(b_q, np.float32)
    W_k, b_k = np.asarray(W_k, np.float32), np.asarray(b_k, np.float32)
    W_v, b_v = np.asarray(W_v, np.float32), np.asarray(b_v, np.float32)
    W_o, b_o = np.asarray(W_o, np.float32), np.asarray(b_o, np.float32)
    gate = np.asarray(gate, np.float32)

    with_mask = bool(np.any(attn_mask))
    nc = _get_nc(with_mask)
    in_maps = build_in_maps(hidden_states, attn_mask, W_q, b_q, W_k, b_k,
                            W_v, b_v, W_o, b_o, gate, with_mask)

    res = run_bass_kernel_spmd(nc, in_maps, core_ids=list(range(NCORES)))
    if res.exec_time_ns is not None:
        print(f"HW exec time: {res.exec_time_ns} ns")

    out = np.zeros((NT, D), dtype=np.float32)
    for r in res.results:
        out += r["outp"].reshape(NT, D).astype(np.float32)
    b_eff = (np.clip(gate, 0.0, 1.0)[:, None] * b_o).sum(axis=0) / H
    out += b_eff[None, :]
    return out.reshape(B, T, D)
